# revision 25
# baseline (speedup 1.0000x reference)
"""Trainium2 Bass kernel for MultiHeadLatentAttention (MLA), 8-core SPMD.

Sharding: data-parallel over batch (4) x tensor-parallel over heads (2).
Core c handles batch c//2 and heads (c%2)*8 .. +8. Each core computes its
partial output projection; the host sums the two TP partials per batch and
adds the (v-bias-folded) output bias.

Device layout is feature-on-partition / token-on-free throughout, so every
projection is a plain matmul chain with no transposes. Attention uses
transposed scores (keys on partitions) so probs feed the AV matmul directly.

v3 notes (vs the 350us v2):
- BOTH down-projections are token-split across the TP pair: each core
  computes all output chunks for its own 512 tokens only. kv-down PE work
  halves; per-core X is 2MB instead of 4MB so the first matmul fires ~4us in.
- three pipelined AllGathers: kv latent + kRot early (hidden under the qd
  chains), then the raw q latent in two 6-chunk halves (hidden under
  kNope/v, feeding the qu chains just in time).
- q sum-of-squares pre-accumulated on the DVE (2 ones-matmuls instead of 24)
- q rot-half via gpsimd DMA block swaps instead of PE permutation matmuls
- v2 carry-overs: rsqrt via Sqrt+fast-reciprocal, head-pair score pipelining,
  v-bias folded into bo on the host, 128x128 universal triangle mask.
"""

import sys
from contextlib import ExitStack

import numpy as np
import ml_dtypes

for _p in ("/opt/trn_rl_repo", "/root/.axon_site/_ro/trn_rl_repo"):
    if _p not in sys.path:
        sys.path.append(_p)

import concourse.bass as bass  # noqa: E402
import concourse.mybir as mybir  # noqa: E402
from concourse import bacc  # noqa: E402
from concourse.bass_utils import run_bass_kernel_spmd  # noqa: E402
from concourse.tile import TileContext  # noqa: E402

# Problem shapes (hardcoded per contract)
B, S, D = 4, 1024, 2048
H = 16
QL, KVL = 1536, 512
NOPE, ROPE, VH = 128, 64, 128
QKH = NOPE + ROPE  # 192
EPS = 1e-6

P = 128
T = S          # tokens per core (one batch)
TH = T // 2    # own-token half per core
DC = D // P    # 16 X chunks
QC = QL // P   # 12 q-latent chunks
KC = KVL // P  # 4 kv-latent chunks
HH = H // 2    # 8 heads per core
NKV = KVL + ROPE  # 576
NEG = -1.0e4   # mask bias (exp underflows to exactly 0)

f32 = mybir.dt.float32
bf16 = mybir.dt.bfloat16
AF = mybir.ActivationFunctionType


def build_nc(start: int):
    nc = bacc.Bacc(None, target_bir_lowering=False, debug=False)

    # all weights arrive host-swizzled to partition-major tile layout
    # [P, tile, c, m] so every DMA descriptor is 1-4KB contiguous
    dp = nc.declare_dram_parameter
    xt = dp("xt", [P, DC * TH], bf16, isOutput=False)     # X[b].T own half
    wqd = dp("wqd", [P, QC * DC * P], bf16, isOutput=False)
    wkvd = dp("wkvd", [P, 4 * DC * P], bf16, isOutput=False)
    wkv5 = dp("wkv5", [P, DC * ROPE], bf16, isOutput=False)
    wqu = dp("wqu", [P, QC * QC * P], bf16, isOutput=False)
    wkn = dp("wkn", [P, HH * KC * P], bf16, isOutput=False)
    wv = dp("wv", [P, 4 * KC * 256], bf16, isOutput=False)
    wo = dp("wo", [P, DC * HH * P], bf16, isOutput=False)
    bqd_i = dp("bqd", [P, QC], f32, isOutput=False)       # qd bias (12 chnk)
    bkvd_i = dp("bkvd", [P, 5], f32, isOutput=False)      # kv down bias
    bqu_i = dp("bqu", [P, QC], f32, isOutput=False)       # perm + scale
    bkvuk = dp("bkvuk", [P, HH], f32, isOutput=False)     # kNope part
    cos2 = dp("cos2", [P, T], bf16, isOutput=False)       # q rope, dup rows
    sina = dp("sina", [P, T], bf16, isOutput=False)       # sign-folded sin
    cosk = dp("cosk", [ROPE, TH], bf16, isOutput=False)   # k rope own tokens
    sink = dp("sink", [ROPE, TH], bf16, isOutput=False)   # sign-folded
    tri_i = dp("tri", [P, P], bf16, isOutput=False)       # diag-band mask
    outt = dp("outt", [D, T], f32, isOutput=True)

    # collectives: kv latent+kRot early, q latent in two halves
    cc1_in = nc.dram_tensor("cc1_in", [5 * P, TH], bf16)
    cc1_out = nc.dram_tensor("cc1_out", [10 * P, TH], bf16)
    cc2a_in = nc.dram_tensor("cc2a_in", [6 * P, TH], bf16)
    cc2a_out = nc.dram_tensor("cc2a_out", [12 * P, TH], bf16)
    cc2b_in = nc.dram_tensor("cc2b_in", [6 * P, TH], bf16)
    cc2b_out = nc.dram_tensor("cc2b_out", [12 * P, TH], bf16)
    RG = [[0, 1], [2, 3], [4, 5], [6, 7]]

    xt_r = xt.rearrange("p (c t) -> p c t", c=DC)
    wqd_r = wqd.rearrange("p (n c m) -> p n c m", n=QC, c=DC)
    wkvd_r = wkvd.rearrange("p (n c m) -> p n c m", n=4, c=DC)
    wkv5_r = wkv5.rearrange("p (c m) -> p c m", c=DC)
    wqu_r = wqu.rearrange("p (n c m) -> p n c m", n=QC, c=QC)
    wkn_r = wkn.rearrange("p (n c m) -> p n c m", n=HH, c=KC)
    wv_r = wv.rearrange("p (n c m) -> p n c m", n=4, c=KC)
    wo_r = wo.rearrange("p (n c m) -> p n c m", n=DC, c=HH)
    outt_r = outt.rearrange("(c p) t -> p c t", p=P)
    cc1_in_r = cc1_in.rearrange("(c p) t -> p c t", p=P)
    cc1_out_r = cc1_out.rearrange("(r c p) t -> p r c t", p=P, r=2)
    cc2a_in_r = cc2a_in.rearrange("(c p) t -> p c t", p=P)
    cc2a_out_r = cc2a_out.rearrange("(r c p) t -> p r c t", p=P, r=2)
    cc2b_in_r = cc2b_in.rearrange("(c p) t -> p c t", p=P)
    cc2b_out_r = cc2b_out.rearrange("(r c p) t -> p r c t", p=P, r=2)

    with TileContext(nc) as tc, ExitStack() as stk:
        const = stk.enter_context(tc.tile_pool(name="const", bufs=1))
        persist = stk.enter_context(tc.tile_pool(name="persist", bufs=1))

        # ---- constants in SBUF ----
        c_bqd = const.tile([P, QC], f32)
        c_bkvd = const.tile([P, 5], f32)
        c_bqu = const.tile([P, QC], f32)
        c_bkvuk = const.tile([P, HH], f32)
        c_tri = const.tile([P, P], bf16)
        c_cos = const.tile([P, T], bf16)
        c_sin = const.tile([P, T], bf16)
        c_cosk = const.tile([ROPE, TH], bf16)
        c_sink = const.tile([ROPE, TH], bf16)
        ones_bf = const.tile([P, P], bf16)
        nc.vector.memset(ones_bf[:], 1.0)
        eps_c = const.tile([P, 1], f32)
        nc.vector.memset(eps_c[:], EPS)

        # ---- persistent activations ----
        t_q = persist.tile([P, QC, T], bf16)      # q heads (nope 0-7, rope+)
        t_kn = persist.tile([P, HH, T], bf16)     # kNope[feat, head, tok]
        t_v = persist.tile([P, T // P, HH * P], bf16)  # v[tok, tchunk, hv]
        t_kr = persist.tile([P, T], bf16)         # kRot full, rows dup
        t_ao = persist.tile([P, HH, T], bf16)     # attn out [vh, head, tok]
        rq = persist.tile([P, T], f32)            # q rms scale (per token)

        # ====== phases 1+2: projections ======
        with tc.tile_pool(name="ph1", bufs=1) as ph1, \
             tc.tile_pool(name="wstream", bufs=2) as wst, \
             tc.tile_pool(name="wqu_p", bufs=2) as wqp, \
             tc.tile_pool(name="wkvu_p", bufs=2) as wkp, \
             tc.tile_pool(name="tmp", bufs=2) as tmp, \
             tc.tile_pool(name="psA", bufs=6, space="PSUM") as psA, \
             tc.tile_pool(name="psR", bufs=1, space="PSUM") as psR:

            t_x = ph1.tile([P, DC, TH], bf16, name="t_x")
            t_kvL = ph1.tile([P, 5, TH], bf16, name="t_kvL")   # local kv
            t_qdL = ph1.tile([P, QC, TH], bf16, name="t_qdL")  # local q lat
            t_krL = ph1.tile([P, TH], bf16, name="t_krL")      # local kRot
            t_kv = ph1.tile([P, KC, T], bf16, name="t_kv")     # gathered kv
            t_qd = ph1.tile([P, QC, T], bf16, name="t_qd")     # gathered q
            rkv = ph1.tile([P, TH], f32, name="rkv")
            acc = ph1.tile([P, 2, TH], bf16, name="acc")       # q ss preacc

            # X chunks + first kv weights lead. gpsimd carries ONLY the
            # collective traffic (+tiny consts) so cc1 fires asap.
            w_kvd = []
            for m in range(4):
                w_kvd.append(wst.tile([P, DC, P], bf16, tag="wkvd", bufs=4,
                                      name="w_kvd"))
            wt5 = wst.tile([P, DC, ROPE], bf16, tag="wkv5", bufs=1,
                           name="wt5")

            def wload(eng, wt, view, pieces=4):
                nchunk = wt.shape[1]
                step = (nchunk + pieces - 1) // pieces
                for i in range(0, nchunk, step):
                    j = min(i + step, nchunk)
                    eng.dma_start(wt[:, i:j, :], view[:, i:j, :])

            # issue order per queue == arrival order. gpsimd gets the early
            # X chunks + consts only, so the cc1 stores aren't head-blocked.
            nc.gpsimd.dma_start(c_bqd[:], bqd_i[:])
            nc.gpsimd.dma_start(c_bkvd[:], bkvd_i[:])
            nc.gpsimd.dma_start(c_cosk[:], cosk[:])
            nc.gpsimd.dma_start(c_sink[:], sink[:])
            wload(nc.scalar, w_kvd[0], wkvd_r[:, 0])
            nc.sync.dma_start(t_x[:, 0:2, :], xt_r[:, 0:2, :])
            nc.gpsimd.dma_start(t_x[:, 2:4, :], xt_r[:, 2:4, :])
            wload(nc.sync, w_kvd[1], wkvd_r[:, 1])
            nc.scalar.dma_start(t_x[:, 4:6, :], xt_r[:, 4:6, :])
            nc.gpsimd.dma_start(t_x[:, 6:8, :], xt_r[:, 6:8, :])
            wload(nc.scalar, w_kvd[2], wkvd_r[:, 2])
            nc.sync.dma_start(t_x[:, 8:10, :], xt_r[:, 8:10, :])
            nc.gpsimd.dma_start(t_x[:, 10:12, :], xt_r[:, 10:12, :])
            wload(nc.sync, w_kvd[3], wkvd_r[:, 3])
            nc.scalar.dma_start(t_x[:, 12:14, :], xt_r[:, 12:14, :])
            nc.gpsimd.dma_start(t_x[:, 14:16, :], xt_r[:, 14:16, :])
            nc.sync.dma_start(wt5[:, :, :ROPE], wkv5_r[:])
            nc.gpsimd.dma_start(c_bqu[:], bqu_i[:])
            nc.gpsimd.dma_start(c_bkvuk[:], bkvuk[:])
            nc.gpsimd.dma_start(c_tri[:], tri_i[:])
            nc.scalar.dma_start(c_cos[:], cos2[:])
            nc.scalar.dma_start(c_sin[:], sina[:])

            def down_chain(wt, m_rows, bias_t, bcol, out_ap):
                # out[m_rows, TH] = wt.T @ X_own + bias
                ps = psA.tile([P, TH], f32, tag="ev", name="ps_ev")
                psm = ps[:m_rows, :]
                for c in range(DC):
                    nc.tensor.matmul(
                        psm, wt[:, c, :m_rows], t_x[:, c, :],
                        start=(c == 0), stop=(c == DC - 1),
                    )
                nc.vector.tensor_scalar_add(
                    out=out_ap[:m_rows, :], in0=psm,
                    scalar1=bias_t[:m_rows, bcol:bcol + 1])

            # ---- kv down for own tokens (5 chunks incl. rope) ----
            for m in range(4):
                down_chain(w_kvd[m], P, c_bkvd, m, t_kvL[:, m, :])
            down_chain(wt5, ROPE, c_bkvd, 4, t_kvL[:ROPE, 4, :])

            # kv rms on own tokens (ss via 4 ones-matmuls, then
            # Sqrt + fast-reciprocal + in-place normalize)
            ps_ms = psR.tile([P, 2, TH], f32, tag="ms", name="ps_ms")
            for c in range(KC):
                sq = tmp.tile([P, TH], bf16, tag="sq")
                nc.vector.tensor_mul(sq[:], t_kvL[:, c, :], t_kvL[:, c, :])
                nc.tensor.matmul(ps_ms[:, 0, :], ones_bf[:], sq[:],
                                 start=(c == 0), stop=(c == KC - 1))
            with tc.high_priority():
                nc.scalar.activation(rkv[:], ps_ms[:, 0, :], AF.Sqrt,
                                     bias=eps_c[:], scale=1.0 / KVL)
                nc.vector.reciprocal_approx_fast(out=rkv[:], in_=rkv[:])
                for c in range(KC):
                    nc.vector.tensor_mul(t_kvL[:, c, :], t_kvL[:, c, :],
                                         rkv[:])
                # RoPE on own kPos (unsigned 32-row swap + sign-folded sin)
                swp = tmp.tile([P, TH], bf16, tag="swp", name="swp",
                               bufs=1)[:ROPE, :]
                nc.sync.dma_start(swp[0:32, :], t_kvL[32:64, 4, :])
                nc.sync.dma_start(swp[32:64, :], t_kvL[0:32, 4, :])
                nc.vector.tensor_mul(t_krL[0:ROPE, :], t_kvL[0:ROPE, 4, :],
                                     c_cosk[:])
                nc.vector.tensor_mul(swp[:], swp[:], c_sink[:])
                nc.vector.tensor_add(t_krL[0:ROPE, :], t_krL[0:ROPE, :],
                                     swp[:])
                nc.sync.dma_start(t_krL[ROPE:P, :], t_krL[0:ROPE, :])

                # ---- collective 1: normalized kv latent + kRot ----
                for m in range(4):
                    nc.gpsimd.dma_start(cc1_in_r[:, m, :], t_kvL[:, m, :])
                nc.gpsimd.dma_start(cc1_in_r[:, 4, :], t_krL[:, :])
                nc.gpsimd.collective_compute(
                    "AllGather", mybir.AluOpType.bypass,
                    replica_groups=RG,
                    ins=[cc1_in[:]], outs=[cc1_out[:]],
                )
                for r in range(2):
                    nc.gpsimd.dma_start(t_kv[:, 0:4, bass.ts(r, TH)],
                                        cc1_out_r[:, r, 0:4, :])
                    nc.gpsimd.dma_start(t_kr[:, bass.ts(r, TH)],
                                        cc1_out_r[:, r, 4, :])

            # ---- q down for own tokens: all 12 chunks ----
            w_qd = []
            for m in range(QC):
                wt = wst.tile([P, DC, P], bf16, tag="wqd", bufs=2,
                              name="w_qd")
                eng = nc.scalar if m % 2 == 0 else nc.sync
                wload(eng, wt, wqd_r[:, m])
                w_qd.append(wt)
                down_chain(wt, P, c_bqd, m, t_qdL[:, m, :])
                # exchange raw latent in two 6-chunk halves
                if m == 5:
                    for mm in range(6):
                        nc.gpsimd.dma_start(cc2a_in_r[:, mm, :],
                                            t_qdL[:, mm, :])
                    nc.gpsimd.collective_compute(
                        "AllGather", mybir.AluOpType.bypass,
                        replica_groups=RG,
                        ins=[cc2a_in[:]], outs=[cc2a_out[:]],
                    )
                    for r in range(2):
                        nc.gpsimd.dma_start(t_qd[:, 0:6, bass.ts(r, TH)],
                                            cc2a_out_r[:, r, 0:6, :])
            for mm in range(6, QC):
                nc.gpsimd.dma_start(cc2b_in_r[:, mm - 6, :], t_qdL[:, mm, :])
            nc.gpsimd.collective_compute(
                "AllGather", mybir.AluOpType.bypass,
                replica_groups=RG,
                ins=[cc2b_in[:]], outs=[cc2b_out[:]],
            )
            for r in range(2):
                nc.gpsimd.dma_start(t_qd[:, 6:12, bass.ts(r, TH)],
                                    cc2b_out_r[:, r, 0:6, :])

            # ---- kNope up-projection (bias add on scalar engine) ----
            kn_w = []
            for m in range(HH):
                wt = wkp.tile([P, KC, P], bf16, tag="wkn", bufs=4,
                              name="kn_w")
                nc.sync.dma_start(wt[:], wkn_r[:, m])
                kn_w.append(wt)
            for m in range(HH):
                wt = kn_w[m]
                for tt in range(2):
                    ps = psA.tile([P, TH], f32, tag="ev", name="ps_kn")
                    for c in range(KC):
                        nc.tensor.matmul(
                            ps, wt[:, c, :],
                            t_kv[:, c, bass.ts(tt, TH)],
                            start=(c == 0), stop=(c == KC - 1),
                        )
                    nc.scalar.activation(
                        t_kn[:, m, bass.ts(tt, TH)], ps, AF.Identity,
                        bias=c_bkvuk[:, m:m + 1])

            # ---- v up-projection (token-on-partition) ----
            for gg in range(4):
                wt = wkp.tile([P, KC, 256], bf16, tag="wv")
                (nc.sync if gg % 2 == 0 else nc.scalar).dma_start(
                    wt[:], wv_r[:, gg])
                for tcb in range(8):
                    ps = psA.tile([P, 256], f32, tag="ev", name="ps_v")
                    for c in range(KC):
                        nc.tensor.matmul(
                            ps,
                            t_kv[:, c, bass.ts(tcb, P)],
                            wt[:, c, :],
                            start=(c == 0), stop=(c == KC - 1),
                        )
                    nc.scalar.activation(
                        t_v[:, tcb, bass.ds(gg * 256, 256)], ps, AF.Copy)

            # ---- q rms from the gathered raw latent (DVE pre-accum) ----
            with tc.tile_wait_until(0.085):
                for tt in range(2):
                    hs = bass.ts(tt, TH)
                    for c in range(QC):
                        if c == 0:
                            nc.vector.tensor_mul(
                                acc[:, tt, :], t_qd[:, 0, hs], t_qd[:, 0, hs])
                        else:
                            sq = tmp.tile([P, TH], bf16, tag="sq")
                            nc.vector.tensor_mul(
                                sq[:], t_qd[:, c, hs], t_qd[:, c, hs])
                            nc.vector.tensor_add(
                                acc[:, tt, :], acc[:, tt, :], sq[:])
                ps_mq = psR.tile([P, 2, TH], f32, tag="ms", name="ps_mq")
                for tt in range(2):
                    hs = bass.ts(tt, TH)
                    nc.tensor.matmul(ps_mq[:, tt, :], ones_bf[:],
                                     acc[:, tt, :], start=True, stop=True)
                    nc.scalar.activation(rq[:, hs], ps_mq[:, tt, :],
                                         AF.Sqrt, bias=eps_c[:],
                                         scale=1.0 / QL)
                    nc.vector.reciprocal_approx_fast(out=rq[:, hs],
                                                     in_=rq[:, hs])

            # ---- q up-projection ----
            # post-processing of chunk m's psums is issued after chunk m+1's
            # matmul chains, so the PE never waits on the DVE stage tiles
            def qu_post(m, ps, tt):
                tsl = bass.ts(tt, TH)
                if m < 8:
                    qsb = tmp.tile([P, TH], bf16, tag="qsb", bufs=2)
                    nc.vector.tensor_mul(qsb[:], ps, rq[:, tsl])
                    nc.scalar.activation(
                        t_q[:, m, tsl], qsb, AF.Identity,
                        bias=c_bqu[:, m:m + 1],
                    )
                else:
                    sq = tmp.tile([P, TH], bf16, tag="ropestage",
                                  bufs=2)
                    nc.vector.tensor_mul(sq[:], ps, rq[:, tsl])
                    nc.vector.tensor_scalar_add(
                        out=sq[:], in0=sq, scalar1=c_bqu[:, m:m + 1],
                    )
                    # rotate-half via gpsimd DMA 32-row block swaps
                    swb = tmp.tile([P, TH], bf16, tag="ropeswap",
                                   bufs=2)
                    nc.gpsimd.dma_start(swb[0:32, :], sq[32:64, :])
                    nc.gpsimd.dma_start(swb[32:64, :], sq[0:32, :])
                    nc.gpsimd.dma_start(swb[64:96, :], sq[96:128, :])
                    nc.gpsimd.dma_start(swb[96:128, :], sq[64:96, :])
                    qc = tmp.tile([P, TH], bf16, tag="ropecos", bufs=2)
                    nc.vector.tensor_mul(qc[:], sq[:], c_cos[:, tsl])
                    nc.vector.tensor_mul(swb[:], swb[:], c_sin[:, tsl])
                    nc.vector.tensor_add(t_q[:, m, tsl], qc[:], swb[:])

            pend = None
            for m in (8, 0, 1, 9, 2, 3, 10, 4, 5, 11, 6, 7):
                wt = wqp.tile([P, QC, P], bf16, tag="wqu")
                eng = nc.scalar if m % 2 == 0 else nc.sync
                wload(eng, wt, wqu_r[:, m])
                cur = []
                for tt in range(2):
                    tsl = bass.ts(tt, TH)
                    ps = psA.tile([P, TH], f32, tag="ev", name="ps_qu")
                    for c in range(QC):
                        nc.tensor.matmul(
                            ps, wt[:, c, :], t_qd[:, c, tsl],
                            start=(c == 0), stop=(c == QC - 1),
                        )
                    cur.append(ps)
                if pend is not None:
                    pm, pps = pend
                    for tt in range(2):
                        qu_post(pm, pps[tt], tt)
                pend = (m, cur)
            pm, pps = pend
            for tt in range(2):
                qu_post(pm, pps[tt], tt)

        # ====== phase 3: attention (transposed scores, max-free) ======
        def vis_kcs(qt):
            return [kc for kc in range(8)
                    if qt * TH + TH - 1 >= kc * P - start]

        with tc.tile_pool(name="att", bufs=2) as att, \
             tc.tile_pool(name="psS", bufs=2, space="PSUM") as psS, \
             tc.tile_pool(name="psD", bufs=1, space="PSUM") as psD, \
             tc.tile_pool(name="psU", bufs=2, space="PSUM") as psU:

            def scores_qt(hp, expts2, qt, kcs=None):
                # expts2 [P, head2, kc, q] for heads (2hp, 2hp+1)
                rc = 8 + hp
                for kc in (vis_kcs(qt) if kcs is None else kcs):
                    lo = max(qt * TH, kc * P - start)
                    w = qt * TH + TH - lo
                    rel = lo - qt * TH
                    sc2 = psS.tile([P, 2, TH], f32, tag="sc", name="sc2")
                    for h2 in range(2):
                        h = 2 * hp + h2
                        nc.tensor.matmul(
                            sc2[:, h2, rel:],
                            t_kn[:, h, bass.ts(kc, P)],
                            t_q[:, h, bass.ds(lo, w)],
                            start=True, stop=False,
                        )
                    for h2 in range(2):
                        r0 = h2 * ROPE
                        nc.tensor.matmul(
                            sc2[:, h2, rel:],
                            t_kr[r0:r0 + ROPE, bass.ts(kc, P)],
                            t_q[r0:r0 + ROPE, rc, bass.ds(lo, w)],
                            start=False, stop=True,
                        )
                    # partially-masked diagonal band
                    b_lo = max(lo, kc * P - start)
                    b_hi = min(qt * TH + TH, kc * P - start + P)
                    bw = b_hi - b_lo
                    if bw > 0:
                        j0 = b_lo - (kc * P - start)
                        br = b_lo - qt * TH
                        for h2 in range(2):
                            nc.vector.tensor_add(
                                sc2[:, h2, br:br + bw],
                                sc2[:, h2, br:br + bw],
                                c_tri[:, j0:j0 + bw])
                    nc.scalar.activation(
                        expts2[:, :, kc, bass.ds(lo, w)],
                        sc2[:, :, rel:], AF.Exp)

            def den_head(hp, expts2, h2):
                den2 = psD.tile([P, 2, TH], f32, name="den2")
                for qt in range(2):
                    kcs = vis_kcs(qt)
                    for i, kc in enumerate(kcs):
                        lo = max(qt * TH, kc * P - start)
                        rel = lo - qt * TH
                        nc.tensor.matmul(
                            den2[:, qt, rel:], ones_bf[:],
                            expts2[:, h2, kc, bass.ds(lo, TH - rel)],
                            start=(i == 0), stop=(i == len(kcs) - 1),
                        )
                rcp = att.tile([P, 2, TH], f32, tag="rcp", name="rcp")
                nc.vector.reciprocal_approx_fast(
                    out=rcp[:, :, :], in_=den2[:, :, :])
                return rcp

            def outU_head(hp, expts2, h2, rcp):
                h = 2 * hp + h2
                for qt in range(2):
                    kcs = vis_kcs(qt)
                    outU = psU.tile([P, TH], f32, tag="outU", name="outU")
                    for i, kc in enumerate(kcs):
                        lo = max(qt * TH, kc * P - start)
                        rel = lo - qt * TH
                        nc.tensor.matmul(
                            outU[:, rel:], t_v[:, kc, bass.ts(h, P)],
                            expts2[:, h2, kc, bass.ds(lo, TH - rel)],
                            start=(i == 0), stop=(i == len(kcs) - 1),
                        )
                    nc.vector.tensor_mul(
                        t_ao[:, h, bass.ts(qt, TH)], outU[:],
                        rcp[:, qt, :])

            # interleave hp-1's den/outU chains between hp's score bursts
            # so the PE has filler while the exp stream catches up
            prev = None
            for hp in range(4):
                cur = att.tile([P, 2, 8, T], bf16, tag="expt", name="expt2")
                scores_qt(hp, cur, 0)
                if prev is not None:
                    rcp0 = den_head(hp - 1, prev, 0)
                scores_qt(hp, cur, 1, kcs=[0, 1, 2, 3])
                if prev is not None:
                    outU_head(hp - 1, prev, 0, rcp0)
                scores_qt(hp, cur, 1, kcs=[4, 5, 6, 7])
                if prev is not None:
                    rcp1 = den_head(hp - 1, prev, 1)
                    outU_head(hp - 1, prev, 1, rcp1)
                prev = cur
            for h2 in range(2):
                rcpt = den_head(3, prev, h2)
                outU_head(3, prev, h2, rcpt)

            # ====== phase 4: output projection ======
            for m in range(DC):
                wt = att.tile([P, HH, P], bf16, tag="wo", name="wo_t",
                              bufs=4)
                eng = nc.gpsimd if m % 2 == 0 else nc.sync
                eng.dma_start(wt[:], wo_r[:, m])
                for tt in range(2):
                    ps = psU.tile([P, TH], f32, tag="outU", name="ps_o")
                    for c in range(HH):
                        nc.tensor.matmul(
                            ps, wt[:, c, :], t_ao[:, c, bass.ts(tt, TH)],
                            start=(c == 0), stop=(c == HH - 1),
                        )
                    ot = att.tile([P, TH], f32, tag="ot", name="ot",
                                  bufs=3)
                    nc.vector.tensor_copy(ot[:], ps)
                    nc.sync.dma_start(outt_r[:, m, bass.ts(tt, TH)], ot[:])

    nc.compile()
    return nc


_CACHE = {}


def _get_nc(start: int):
    if start not in _CACHE:
        _CACHE[start] = build_nc(start)
    return _CACHE[start]


def _prep_inputs(X, base_freq, Wqd, bqd, gq, Wqu, bqu, Wkv, bkv, gkv,
                 Wkvu, bkvu, Wo, bo, start):
    f = np.float32
    X = np.asarray(X, f)
    base_freq = np.asarray(base_freq, f)
    Wqd = np.asarray(Wqd, f); bqd = np.asarray(bqd, f)
    gq = np.asarray(gq, f); Wqu = np.asarray(Wqu, f); bqu = np.asarray(bqu, f)
    Wkv = np.asarray(Wkv, f); bkv = np.asarray(bkv, f)
    gkv = np.asarray(gkv, f); Wkvu = np.asarray(Wkvu, f)
    bkvu = np.asarray(bkvu, f)
    Wo = np.asarray(Wo, f); bo = np.asarray(bo, f)
    start = int(np.asarray(start).item())
    assert start >= 0

    scale = QKH ** (-0.5)
    bf = ml_dtypes.bfloat16

    # v-bias exact fold: probs sum to 1, so the v bias contributes
    # Wo @ bv to every token's output.
    bv = bkvu.reshape(H, NOPE + VH)[:, NOPE:].reshape(H * VH)
    bo_eff = bo + Wo @ bv

    def _sw(wt, nt, c, m):
        # [c*P, nt*m] -> partition-major tiles [P, nt*c*m]
        a = np.asarray(wt, f).reshape(c, P, nt, m)
        a = np.ascontiguousarray(a.transpose(1, 2, 0, 3)).astype(bf)
        return a.reshape(P, nt * c * m)

    # full down-projection weights (token-split: same on both cores)
    wqd_t = _sw(Wqd.T, QC, DC, P)
    bqd_t = np.ascontiguousarray(bqd.reshape(QC, P).T)        # (P, 12)
    wkv_t = Wkv.T.astype(f)                                   # (D, NKV)
    wkvd = _sw(wkv_t[:, :512], 4, DC, P)
    wkv5 = _sw(wkv_t[:, 512:576], 1, DC, ROPE)
    bkvd_p = np.zeros((5 * P,), f); bkvd_p[:NKV] = bkv
    bkvd = np.ascontiguousarray(bkvd_p.reshape(5, P).T)

    ang = base_freq[:S]                                       # (S, ROPE)
    cos = np.ascontiguousarray(np.cos(ang).T.astype(f))       # (ROPE, S)
    sin = np.ascontiguousarray(np.sin(ang).T.astype(f))
    cos2 = np.ascontiguousarray(
        np.concatenate([cos, cos], 0)).astype(bf)             # (128, S)
    sgn = np.ones((ROPE, 1), f); sgn[:ROPE // 2] = -1.0
    sins = sin * sgn                                          # sign-folded
    sina = np.ascontiguousarray(np.concatenate([sins, sins], 0)).astype(bf)

    # universal diagonal-band mask: for the block at k = kc*P + p,
    # q = (kc*P - start) + j, visibility is p <= j.
    pp = np.arange(P)
    tri = np.where(pp[:, None] <= pp[None, :], 0.0, NEG).astype(bf)
    tri = np.ascontiguousarray(tri)

    # per head-group tensors
    perm_q = np.concatenate(
        [np.arange(h * QKH, h * QKH + NOPE) for h in range(HH)]
        + [np.arange(h * QKH + NOPE, (h + 1) * QKH) for h in range(HH)]
    )
    perm_kv = np.concatenate(
        [np.arange(h * (NOPE + VH), h * (NOPE + VH) + NOPE) for h in range(HH)]
        + [np.arange(h * (NOPE + VH) + NOPE, (h + 1) * (NOPE + VH))
           for h in range(HH)]
    )
    gmaps = []
    for g in range(2):
        rq_ = slice(g * HH * QKH, (g + 1) * HH * QKH)
        rkv_ = slice(g * HH * (NOPE + VH), (g + 1) * HH * (NOPE + VH))
        wqu_g = (Wqu[rq_, :] * gq[None, :] * scale)[perm_q]   # (1536, QL)
        bqu_g = (bqu[rq_] * scale)[perm_q]
        wkvu_g = (Wkvu[rkv_, :] * gkv[None, :])[perm_kv]      # (2048, KVL)
        bkvu_g = bkvu[rkv_][perm_kv]
        wo_g = Wo[:, g * HH * VH:(g + 1) * HH * VH]           # (D, 1024)
        tg = slice(g * TH, (g + 1) * TH)
        wkvu_t = wkvu_g.T                                     # (KVL, 2048)
        gmaps.append({
            "wqu": _sw(wqu_g.T, QC, QC, P),
            "bqu": np.ascontiguousarray(bqu_g.reshape(QC, P).T),
            "wkn": _sw(wkvu_t[:, :HH * P], HH, KC, P),
            "wv": _sw(wkvu_t[:, HH * P:], 4, KC, 256),
            "bkvuk": np.ascontiguousarray(
                bkvu_g[:HH * NOPE].reshape(HH, P).T),
            "wo": _sw(wo_g.T, DC, HH, P),
            "cosk": np.ascontiguousarray(cos[:, tg]).astype(bf),
            "sink": np.ascontiguousarray(sins[:, tg]).astype(bf),
        })

    xts = [[_sw(X[b].T[:, g * TH:(g + 1) * TH], 1, DC, TH)
            for g in range(2)] for b in range(B)]

    in_maps = []
    for c in range(8):
        b, g = c // 2, c % 2
        m = {
            "xt": xts[b][g], "wqd": wqd_t, "bqd": bqd_t,
            "wkvd": wkvd, "wkv5": wkv5, "bkvd": bkvd,
            "cos2": cos2, "sina": sina, "tri": tri,
        }
        m.update(gmaps[g])
        in_maps.append(m)
    return in_maps, bo_eff, start


def kernel(**inputs) -> np.ndarray:
    in_maps, bo_eff, start = _prep_inputs(**inputs)
    nc = _get_nc(start)
    try:
        res = run_bass_kernel_spmd(nc, in_maps, core_ids=list(range(8)))
    except Exception:
        res = run_bass_kernel_spmd(nc, in_maps, core_ids=list(range(8)))
    out = np.empty((B, S, D), np.float32)
    for b in range(B):
        acc = res.results[2 * b]["outt"] + res.results[2 * b + 1]["outt"]
        out[b] = acc.T + bo_eff[None, :]
    return out


# revision 30
# speedup vs baseline: 1.0187x; 1.0187x over previous
"""Trainium2 Bass kernel for MultiHeadLatentAttention (MLA), 8-core SPMD.

Sharding: data-parallel over batch (4) x tensor-parallel over heads (2).
Core c handles batch c//2 and heads (c%2)*8 .. +8. Each core computes its
partial output projection; the host sums the two TP partials per batch and
adds the (v-bias-folded) output bias.

Device layout is feature-on-partition / token-on-free throughout, so every
projection is a plain matmul chain with no transposes. Attention uses
transposed scores (keys on partitions) so probs feed the AV matmul directly.

v3 notes (vs the 350us v2):
- BOTH down-projections are token-split across the TP pair: each core
  computes all output chunks for its own 512 tokens only. kv-down PE work
  halves; per-core X is 2MB instead of 4MB so the first matmul fires ~4us in.
- three pipelined AllGathers: kv latent + kRot early (hidden under the qd
  chains), then the raw q latent in two 6-chunk halves (hidden under
  kNope/v, feeding the qu chains just in time).
- q sum-of-squares pre-accumulated on the DVE (2 ones-matmuls instead of 24)
- q rot-half via gpsimd DMA block swaps instead of PE permutation matmuls
- v2 carry-overs: rsqrt via Sqrt+fast-reciprocal, head-pair score pipelining,
  v-bias folded into bo on the host, 128x128 universal triangle mask.
"""

import sys
from contextlib import ExitStack

import numpy as np
import ml_dtypes

for _p in ("/opt/trn_rl_repo", "/root/.axon_site/_ro/trn_rl_repo"):
    if _p not in sys.path:
        sys.path.append(_p)

import concourse.bass as bass  # noqa: E402
import concourse.mybir as mybir  # noqa: E402
from concourse import bacc  # noqa: E402
from concourse.bass_utils import run_bass_kernel_spmd  # noqa: E402
from concourse.tile import TileContext  # noqa: E402

# Problem shapes (hardcoded per contract)
B, S, D = 4, 1024, 2048
H = 16
QL, KVL = 1536, 512
NOPE, ROPE, VH = 128, 64, 128
QKH = NOPE + ROPE  # 192
EPS = 1e-6

P = 128
T = S          # tokens per core (one batch)
TH = T // 2    # own-token half per core
DC = D // P    # 16 X chunks
QC = QL // P   # 12 q-latent chunks
KC = KVL // P  # 4 kv-latent chunks
HH = H // 2    # 8 heads per core
NKV = KVL + ROPE  # 576
NEG = -1.0e4   # mask bias (exp underflows to exactly 0)

f32 = mybir.dt.float32
bf16 = mybir.dt.bfloat16
AF = mybir.ActivationFunctionType


def build_nc(start: int):
    nc = bacc.Bacc(None, target_bir_lowering=False, debug=False)

    # all weights arrive host-swizzled to partition-major tile layout
    # [P, tile, c, m] so every DMA descriptor is 1-4KB contiguous
    dp = nc.declare_dram_parameter
    xt = dp("xt", [P, DC * TH], bf16, isOutput=False)     # X[b].T own half
    wqd = dp("wqd", [P, QC * DC * P], bf16, isOutput=False)
    wkvd = dp("wkvd", [P, 4 * DC * P], bf16, isOutput=False)
    wkv5 = dp("wkv5", [P, DC * ROPE], bf16, isOutput=False)
    wqu = dp("wqu", [P, QC * QC * P], bf16, isOutput=False)
    wkn = dp("wkn", [P, HH * KC * P], bf16, isOutput=False)
    wv = dp("wv", [P, 4 * KC * 256], bf16, isOutput=False)
    wo = dp("wo", [P, DC * HH * P], bf16, isOutput=False)
    bqd_i = dp("bqd", [P, QC], f32, isOutput=False)       # qd bias (12 chnk)
    bkvd_i = dp("bkvd", [P, 5], f32, isOutput=False)      # kv down bias
    bqu_i = dp("bqu", [P, QC], f32, isOutput=False)       # perm + scale
    bkvuk = dp("bkvuk", [P, HH], f32, isOutput=False)     # kNope part
    cos2 = dp("cos2", [P, T], bf16, isOutput=False)       # q rope, dup rows
    sina = dp("sina", [P, T], bf16, isOutput=False)       # sign-folded sin
    cosk = dp("cosk", [ROPE, TH], bf16, isOutput=False)   # k rope own tokens
    sink = dp("sink", [ROPE, TH], bf16, isOutput=False)   # sign-folded
    tri_i = dp("tri", [P, P], bf16, isOutput=False)       # diag-band mask
    outt = dp("outt", [D, T], bf16, isOutput=True)

    # collectives: kv latent+kRot early, q latent in two halves
    cc1_in = nc.dram_tensor("cc1_in", [5 * P, TH], bf16)
    cc1_out = nc.dram_tensor("cc1_out", [10 * P, TH], bf16)
    cc2a_in = nc.dram_tensor("cc2a_in", [6 * P, TH], bf16)
    cc2a_out = nc.dram_tensor("cc2a_out", [12 * P, TH], bf16)
    cc2b_in = nc.dram_tensor("cc2b_in", [6 * P, TH], bf16)
    cc2b_out = nc.dram_tensor("cc2b_out", [12 * P, TH], bf16)
    RG = [[0, 1], [2, 3], [4, 5], [6, 7]]

    xt_r = xt.rearrange("p (c t) -> p c t", c=DC)
    wqd_r = wqd.rearrange("p (n c m) -> p n c m", n=QC, c=DC)
    wkvd_r = wkvd.rearrange("p (n c m) -> p n c m", n=4, c=DC)
    wkv5_r = wkv5.rearrange("p (c m) -> p c m", c=DC)
    wqu_r = wqu.rearrange("p (n c m) -> p n c m", n=QC, c=QC)
    wkn_r = wkn.rearrange("p (n c m) -> p n c m", n=HH, c=KC)
    wv_r = wv.rearrange("p (n c m) -> p n c m", n=4, c=KC)
    wo_r = wo.rearrange("p (n c m) -> p n c m", n=DC, c=HH)
    outt_r = outt.rearrange("(c p) t -> p c t", p=P)
    cc1_in_r = cc1_in.rearrange("(c p) t -> p c t", p=P)
    cc1_out_r = cc1_out.rearrange("(r c p) t -> p r c t", p=P, r=2)
    cc2a_in_r = cc2a_in.rearrange("(c p) t -> p c t", p=P)
    cc2a_out_r = cc2a_out.rearrange("(r c p) t -> p r c t", p=P, r=2)
    cc2b_in_r = cc2b_in.rearrange("(c p) t -> p c t", p=P)
    cc2b_out_r = cc2b_out.rearrange("(r c p) t -> p r c t", p=P, r=2)

    with TileContext(nc) as tc, ExitStack() as stk:
        const = stk.enter_context(tc.tile_pool(name="const", bufs=1))
        persist = stk.enter_context(tc.tile_pool(name="persist", bufs=1))

        # ---- constants in SBUF ----
        c_bqd = const.tile([P, QC], f32)
        c_bkvd = const.tile([P, 5], f32)
        c_bqu = const.tile([P, QC], f32)
        c_bkvuk = const.tile([P, HH], f32)
        c_tri = const.tile([P, P], bf16)
        c_cos = const.tile([P, T], bf16)
        c_sin = const.tile([P, T], bf16)
        c_cosk = const.tile([ROPE, TH], bf16)
        c_sink = const.tile([ROPE, TH], bf16)
        ones_bf = const.tile([P, P], bf16)
        nc.vector.memset(ones_bf[:], 1.0)
        eps_c = const.tile([P, 1], f32)
        nc.vector.memset(eps_c[:], EPS)

        # ---- persistent activations ----
        t_q = persist.tile([P, QC, T], bf16)      # q heads (nope 0-7, rope+)
        t_kn = persist.tile([P, HH, T], bf16)     # kNope[feat, head, tok]
        t_v = persist.tile([P, T // P, HH * P], bf16)  # v[tok, tchunk, hv]
        t_kr = persist.tile([P, T], bf16)         # kRot full, rows dup
        t_ao = persist.tile([P, HH, T], bf16)     # attn out [vh, head, tok]
        rq = persist.tile([P, T], f32)            # q rms scale (per token)

        # ====== phases 1+2: projections ======
        with tc.tile_pool(name="ph1", bufs=1) as ph1, \
             tc.tile_pool(name="wstream", bufs=2) as wst, \
             tc.tile_pool(name="wqu_p", bufs=2) as wqp, \
             tc.tile_pool(name="wkvu_p", bufs=2) as wkp, \
             tc.tile_pool(name="tmp", bufs=2) as tmp, \
             tc.tile_pool(name="psA", bufs=6, space="PSUM") as psA, \
             tc.tile_pool(name="psR", bufs=1, space="PSUM") as psR:

            t_x = ph1.tile([P, DC, TH], bf16, name="t_x")
            t_kvL = ph1.tile([P, 5, TH], bf16, name="t_kvL")   # local kv
            t_qdL = ph1.tile([P, QC, TH], bf16, name="t_qdL")  # local q lat
            t_krL = ph1.tile([P, TH], bf16, name="t_krL")      # local kRot
            t_kv = ph1.tile([P, KC, T], bf16, name="t_kv")     # gathered kv
            t_qd = ph1.tile([P, QC, T], bf16, name="t_qd")     # gathered q
            rkv = ph1.tile([P, TH], f32, name="rkv")
            acc = ph1.tile([P, 2, TH], bf16, name="acc")       # q ss preacc

            # X chunks + first kv weights lead. gpsimd carries ONLY the
            # collective traffic (+tiny consts) so cc1 fires asap.
            w_kvd = []
            for m in range(4):
                w_kvd.append(wst.tile([P, DC, P], bf16, tag="wkvd", bufs=4,
                                      name="w_kvd"))
            wt5 = wst.tile([P, DC, ROPE], bf16, tag="wkv5", bufs=1,
                           name="wt5")

            def wload(eng, wt, view, pieces=4):
                nchunk = wt.shape[1]
                step = (nchunk + pieces - 1) // pieces
                for i in range(0, nchunk, step):
                    j = min(i + step, nchunk)
                    eng.dma_start(wt[:, i:j, :], view[:, i:j, :])

            # preload the scalar activation table off the rms critical chain
            nc.scalar.activation(rkv[:, 0:1], eps_c[:], AF.Sqrt,
                                 bias=eps_c[:], scale=1.0)

            # issue order per queue == arrival order. Strict priority:
            # kv-down weights + X first (kvd chains gate collective 1),
            # qd weights stream after, cos/sin tables late.
            nc.gpsimd.dma_start(c_bqd[:], bqd_i[:])
            nc.gpsimd.dma_start(c_bkvd[:], bkvd_i[:])
            nc.gpsimd.dma_start(c_cosk[:], cosk[:])
            nc.gpsimd.dma_start(c_sink[:], sink[:])
            wload(nc.scalar, w_kvd[0], wkvd_r[:, 0])
            nc.sync.dma_start(t_x[:, 0:2, :], xt_r[:, 0:2, :])
            nc.gpsimd.dma_start(t_x[:, 2:4, :], xt_r[:, 2:4, :])
            wload(nc.sync, w_kvd[1], wkvd_r[:, 1])
            nc.scalar.dma_start(t_x[:, 4:6, :], xt_r[:, 4:6, :])
            nc.gpsimd.dma_start(t_x[:, 6:8, :], xt_r[:, 6:8, :])
            wload(nc.gpsimd, w_kvd[2], wkvd_r[:, 2])
            nc.sync.dma_start(t_x[:, 8:10, :], xt_r[:, 8:10, :])
            nc.scalar.dma_start(t_x[:, 12:14, :], xt_r[:, 12:14, :])
            wload(nc.sync, w_kvd[3], wkvd_r[:, 3])
            nc.scalar.dma_start(t_x[:, 14:16, :], xt_r[:, 14:16, :])
            nc.gpsimd.dma_start(t_x[:, 10:12, :], xt_r[:, 10:12, :])
            nc.gpsimd.dma_start(wt5[:, :, :ROPE], wkv5_r[:])
            nc.gpsimd.dma_start(c_bqu[:], bqu_i[:])
            nc.gpsimd.dma_start(c_bkvuk[:], bkvuk[:])
            nc.gpsimd.dma_start(c_tri[:], tri_i[:])

            def down_chain(wt, m_rows, bias_t, bcol, out_ap):
                # out[m_rows, TH] = wt.T @ X_own + bias
                ps = psA.tile([P, TH], f32, tag="ev", name="ps_ev")
                psm = ps[:m_rows, :]
                for c in range(DC):
                    nc.tensor.matmul(
                        psm, wt[:, c, :m_rows], t_x[:, c, :],
                        start=(c == 0), stop=(c == DC - 1),
                    )
                nc.vector.tensor_scalar_add(
                    out=out_ap[:m_rows, :], in0=psm,
                    scalar1=bias_t[:m_rows, bcol:bcol + 1])

            # ---- kv down for own tokens (5 chunks incl. rope) ----
            for m in range(4):
                down_chain(w_kvd[m], P, c_bkvd, m, t_kvL[:, m, :])
            down_chain(wt5, ROPE, c_bkvd, 4, t_kvL[:ROPE, 4, :])

            # kv rms on own tokens (ss via 4 ones-matmuls, then
            # Sqrt + fast-reciprocal + in-place normalize)
            ps_ms = psR.tile([P, 2, TH], f32, tag="ms", name="ps_ms")
            for c in range(KC):
                sq = tmp.tile([P, TH], bf16, tag="sq")
                nc.vector.tensor_mul(sq[:], t_kvL[:, c, :], t_kvL[:, c, :])
                nc.tensor.matmul(ps_ms[:, 0, :], ones_bf[:], sq[:],
                                 start=(c == 0), stop=(c == KC - 1))
            with tc.high_priority():
                nc.scalar.activation(rkv[:], ps_ms[:, 0, :], AF.Sqrt,
                                     bias=eps_c[:], scale=1.0 / KVL)
                nc.vector.reciprocal_approx_fast(out=rkv[:], in_=rkv[:])
                for c in range(KC):
                    nc.vector.tensor_mul(t_kvL[:, c, :], t_kvL[:, c, :],
                                         rkv[:])
                # RoPE on own kPos (unsigned 32-row swap + sign-folded sin)
                swp = tmp.tile([P, TH], bf16, tag="swp", name="swp",
                               bufs=1)[:ROPE, :]
                nc.sync.dma_start(swp[0:32, :], t_kvL[32:64, 4, :])
                nc.sync.dma_start(swp[32:64, :], t_kvL[0:32, 4, :])
                nc.vector.tensor_mul(t_krL[0:ROPE, :], t_kvL[0:ROPE, 4, :],
                                     c_cosk[:])
                nc.vector.tensor_mul(swp[:], swp[:], c_sink[:])
                nc.vector.tensor_add(t_krL[0:ROPE, :], t_krL[0:ROPE, :],
                                     swp[:])
                nc.sync.dma_start(t_krL[ROPE:P, :], t_krL[0:ROPE, :])

                # ---- collective 1: normalized kv latent + kRot ----
                for m in range(4):
                    nc.gpsimd.dma_start(cc1_in_r[:, m, :], t_kvL[:, m, :])
                nc.gpsimd.dma_start(cc1_in_r[:, 4, :], t_krL[:, :])
                nc.gpsimd.collective_compute(
                    "AllGather", mybir.AluOpType.bypass,
                    replica_groups=RG,
                    ins=[cc1_in[:]], outs=[cc1_out[:]],
                )
                for r in range(2):
                    nc.gpsimd.dma_start(t_kv[:, 0:4, bass.ts(r, TH)],
                                        cc1_out_r[:, r, 0:4, :])
                    nc.gpsimd.dma_start(t_kr[:, bass.ts(r, TH)],
                                        cc1_out_r[:, r, 4, :])

            # ---- q down for own tokens: all 12 chunks ----
            w_qd = []
            for m in range(QC):
                wt = wst.tile([P, DC, P], bf16, tag="wqd", bufs=2,
                              name="w_qd")
                eng = nc.scalar if m % 2 == 0 else nc.sync
                wload(eng, wt, wqd_r[:, m])
                w_qd.append(wt)
                down_chain(wt, P, c_bqd, m, t_qdL[:, m, :])
                # exchange raw latent in two 6-chunk halves
                if m == 5:
                    for mm in range(6):
                        nc.gpsimd.dma_start(cc2a_in_r[:, mm, :],
                                            t_qdL[:, mm, :])
                    nc.gpsimd.collective_compute(
                        "AllGather", mybir.AluOpType.bypass,
                        replica_groups=RG,
                        ins=[cc2a_in[:]], outs=[cc2a_out[:]],
                    )
                    for r in range(2):
                        nc.gpsimd.dma_start(t_qd[:, 0:6, bass.ts(r, TH)],
                                            cc2a_out_r[:, r, 0:6, :])
            nc.scalar.dma_start(c_cos[:], cos2[:])
            nc.scalar.dma_start(c_sin[:], sina[:])
            for mm in range(6, QC):
                nc.gpsimd.dma_start(cc2b_in_r[:, mm - 6, :], t_qdL[:, mm, :])
            nc.gpsimd.collective_compute(
                "AllGather", mybir.AluOpType.bypass,
                replica_groups=RG,
                ins=[cc2b_in[:]], outs=[cc2b_out[:]],
            )
            for r in range(2):
                nc.gpsimd.dma_start(t_qd[:, 6:12, bass.ts(r, TH)],
                                    cc2b_out_r[:, r, 0:6, :])

            # ---- kNope up-projection (bias add on scalar engine) ----
            kn_w = []
            for m in range(HH):
                wt = wkp.tile([P, KC, P], bf16, tag="wkn", bufs=4,
                              name="kn_w")
                nc.sync.dma_start(wt[:], wkn_r[:, m])
                kn_w.append(wt)
            for m in range(HH):
                wt = kn_w[m]
                for tt in range(2):
                    ps = psA.tile([P, TH], f32, tag="ev", name="ps_kn")
                    for c in range(KC):
                        nc.tensor.matmul(
                            ps, wt[:, c, :],
                            t_kv[:, c, bass.ts(tt, TH)],
                            start=(c == 0), stop=(c == KC - 1),
                        )
                    nc.scalar.activation(
                        t_kn[:, m, bass.ts(tt, TH)], ps, AF.Identity,
                        bias=c_bkvuk[:, m:m + 1])

            # ---- v up-projection (token-on-partition) ----
            for gg in range(4):
                wt = wkp.tile([P, KC, 256], bf16, tag="wv")
                (nc.sync if gg % 2 == 0 else nc.scalar).dma_start(
                    wt[:], wv_r[:, gg])
                for tcb in range(8):
                    ps = psA.tile([P, 256], f32, tag="ev", name="ps_v")
                    for c in range(KC):
                        nc.tensor.matmul(
                            ps,
                            t_kv[:, c, bass.ts(tcb, P)],
                            wt[:, c, :],
                            start=(c == 0), stop=(c == KC - 1),
                        )
                    nc.scalar.activation(
                        t_v[:, tcb, bass.ds(gg * 256, 256)], ps, AF.Copy)

            # ---- q rms from the gathered raw latent (DVE pre-accum) ----
            with tc.tile_wait_until(0.085):
                for tt in range(2):
                    hs = bass.ts(tt, TH)
                    for c in range(QC):
                        if c == 0:
                            nc.vector.tensor_mul(
                                acc[:, tt, :], t_qd[:, 0, hs], t_qd[:, 0, hs])
                        else:
                            sq = tmp.tile([P, TH], bf16, tag="sq")
                            nc.vector.tensor_mul(
                                sq[:], t_qd[:, c, hs], t_qd[:, c, hs])
                            nc.vector.tensor_add(
                                acc[:, tt, :], acc[:, tt, :], sq[:])
                ps_mq = psR.tile([P, 2, TH], f32, tag="ms", name="ps_mq")
                for tt in range(2):
                    hs = bass.ts(tt, TH)
                    nc.tensor.matmul(ps_mq[:, tt, :], ones_bf[:],
                                     acc[:, tt, :], start=True, stop=True)
                    nc.scalar.activation(rq[:, hs], ps_mq[:, tt, :],
                                         AF.Sqrt, bias=eps_c[:],
                                         scale=1.0 / QL)
                    nc.vector.reciprocal_approx_fast(out=rq[:, hs],
                                                     in_=rq[:, hs])

            # ---- q up-projection ----
            # post-processing of chunk m's psums is issued after chunk m+1's
            # matmul chains, so the PE never waits on the DVE stage tiles
            def qu_post(m, ps, tt):
                tsl = bass.ts(tt, TH)
                if m < 8:
                    qsb = tmp.tile([P, TH], bf16, tag="qsb", bufs=2)
                    nc.vector.tensor_mul(qsb[:], ps, rq[:, tsl])
                    nc.scalar.activation(
                        t_q[:, m, tsl], qsb, AF.Identity,
                        bias=c_bqu[:, m:m + 1],
                    )
                else:
                    sq = tmp.tile([P, TH], bf16, tag="ropestage",
                                  bufs=2)
                    nc.vector.tensor_mul(sq[:], ps, rq[:, tsl])
                    nc.vector.tensor_scalar_add(
                        out=sq[:], in0=sq, scalar1=c_bqu[:, m:m + 1],
                    )
                    # rotate-half via gpsimd DMA 32-row block swaps
                    swb = tmp.tile([P, TH], bf16, tag="ropeswap",
                                   bufs=2)
                    nc.gpsimd.dma_start(swb[0:32, :], sq[32:64, :])
                    nc.gpsimd.dma_start(swb[32:64, :], sq[0:32, :])
                    nc.gpsimd.dma_start(swb[64:96, :], sq[96:128, :])
                    nc.gpsimd.dma_start(swb[96:128, :], sq[64:96, :])
                    qc = tmp.tile([P, TH], bf16, tag="ropecos", bufs=2)
                    nc.vector.tensor_mul(qc[:], sq[:], c_cos[:, tsl])
                    nc.vector.tensor_mul(swb[:], swb[:], c_sin[:, tsl])
                    nc.vector.tensor_add(t_q[:, m, tsl], qc[:], swb[:])

            pend = None
            for m in (8, 0, 1, 9, 2, 3, 10, 4, 5, 11, 6, 7):
                wt = wqp.tile([P, QC, P], bf16, tag="wqu")
                eng = nc.scalar if m % 2 == 0 else nc.sync
                wload(eng, wt, wqu_r[:, m])
                cur = []
                for tt in range(2):
                    tsl = bass.ts(tt, TH)
                    ps = psA.tile([P, TH], f32, tag="ev", name="ps_qu")
                    for c in range(QC):
                        nc.tensor.matmul(
                            ps, wt[:, c, :], t_qd[:, c, tsl],
                            start=(c == 0), stop=(c == QC - 1),
                        )
                    cur.append(ps)
                if pend is not None:
                    pm, pps = pend
                    for tt in range(2):
                        qu_post(pm, pps[tt], tt)
                pend = (m, cur)
            pm, pps = pend
            for tt in range(2):
                qu_post(pm, pps[tt], tt)

        # ====== phase 3: attention (transposed scores, max-free) ======
        def vis_kcs(qt):
            return [kc for kc in range(8)
                    if qt * TH + TH - 1 >= kc * P - start]

        with tc.tile_pool(name="att", bufs=2) as att, \
             tc.tile_pool(name="psS", bufs=2, space="PSUM") as psS, \
             tc.tile_pool(name="psD", bufs=1, space="PSUM") as psD, \
             tc.tile_pool(name="psU", bufs=2, space="PSUM") as psU:

            def scores_qt(hp, expts2, qt, kcs=None):
                # expts2 [P, head2, kc, q] for heads (2hp, 2hp+1)
                rc = 8 + hp
                for kc in (vis_kcs(qt) if kcs is None else kcs):
                    lo = max(qt * TH, kc * P - start)
                    w = qt * TH + TH - lo
                    rel = lo - qt * TH
                    sc2 = psS.tile([P, 2, TH], f32, tag="sc", name="sc2")
                    for h2 in range(2):
                        h = 2 * hp + h2
                        nc.tensor.matmul(
                            sc2[:, h2, rel:],
                            t_kn[:, h, bass.ts(kc, P)],
                            t_q[:, h, bass.ds(lo, w)],
                            start=True, stop=False,
                        )
                    for h2 in range(2):
                        r0 = h2 * ROPE
                        nc.tensor.matmul(
                            sc2[:, h2, rel:],
                            t_kr[r0:r0 + ROPE, bass.ts(kc, P)],
                            t_q[r0:r0 + ROPE, rc, bass.ds(lo, w)],
                            start=False, stop=True,
                        )
                    # partially-masked diagonal band
                    b_lo = max(lo, kc * P - start)
                    b_hi = min(qt * TH + TH, kc * P - start + P)
                    bw = b_hi - b_lo
                    if bw > 0:
                        j0 = b_lo - (kc * P - start)
                        br = b_lo - qt * TH
                        for h2 in range(2):
                            nc.vector.tensor_add(
                                sc2[:, h2, br:br + bw],
                                sc2[:, h2, br:br + bw],
                                c_tri[:, j0:j0 + bw])
                    nc.scalar.activation(
                        expts2[:, :, kc, bass.ds(lo, w)],
                        sc2[:, :, rel:], AF.Exp)

            def den_head(hp, expts2, h2):
                den2 = psD.tile([P, 2, TH], f32, name="den2")
                for qt in range(2):
                    kcs = vis_kcs(qt)
                    for i, kc in enumerate(kcs):
                        lo = max(qt * TH, kc * P - start)
                        rel = lo - qt * TH
                        nc.tensor.matmul(
                            den2[:, qt, rel:], ones_bf[:],
                            expts2[:, h2, kc, bass.ds(lo, TH - rel)],
                            start=(i == 0), stop=(i == len(kcs) - 1),
                        )
                rcp = att.tile([P, 2, TH], f32, tag="rcp", name="rcp")
                nc.vector.reciprocal_approx_fast(
                    out=rcp[:, :, :], in_=den2[:, :, :])
                return rcp

            def outU_head(hp, expts2, h2, rcp):
                h = 2 * hp + h2
                for qt in range(2):
                    kcs = vis_kcs(qt)
                    outU = psU.tile([P, TH], f32, tag="outU", name="outU")
                    for i, kc in enumerate(kcs):
                        lo = max(qt * TH, kc * P - start)
                        rel = lo - qt * TH
                        nc.tensor.matmul(
                            outU[:, rel:], t_v[:, kc, bass.ts(h, P)],
                            expts2[:, h2, kc, bass.ds(lo, TH - rel)],
                            start=(i == 0), stop=(i == len(kcs) - 1),
                        )
                    nc.vector.tensor_mul(
                        t_ao[:, h, bass.ts(qt, TH)], outU[:],
                        rcp[:, qt, :])

            # interleave hp-1's den/outU chains between hp's score bursts
            # so the PE has filler while the exp stream catches up
            prev = None
            for hp in range(4):
                cur = att.tile([P, 2, 8, T], bf16, tag="expt", name="expt2")
                scores_qt(hp, cur, 0)
                if prev is not None:
                    rcp0 = den_head(hp - 1, prev, 0)
                scores_qt(hp, cur, 1, kcs=[0, 1, 2, 3])
                if prev is not None:
                    outU_head(hp - 1, prev, 0, rcp0)
                scores_qt(hp, cur, 1, kcs=[4, 5, 6, 7])
                if prev is not None:
                    rcp1 = den_head(hp - 1, prev, 1)
                    outU_head(hp - 1, prev, 1, rcp1)
                prev = cur
            for h2 in range(2):
                rcpt = den_head(3, prev, h2)
                outU_head(3, prev, h2, rcpt)

            # ====== phase 4: output projection ======
            for m in range(DC):
                wt = att.tile([P, HH, P], bf16, tag="wo", name="wo_t",
                              bufs=4)
                eng = nc.gpsimd if m % 2 == 0 else nc.sync
                eng.dma_start(wt[:], wo_r[:, m])
                for tt in range(2):
                    ps = psU.tile([P, TH], f32, tag="outU", name="ps_o")
                    for c in range(HH):
                        nc.tensor.matmul(
                            ps, wt[:, c, :], t_ao[:, c, bass.ts(tt, TH)],
                            start=(c == 0), stop=(c == HH - 1),
                        )
                    ot = att.tile([P, TH], bf16, tag="ot", name="ot",
                                  bufs=3)
                    nc.vector.tensor_copy(ot[:], ps)
                    nc.sync.dma_start(outt_r[:, m, bass.ts(tt, TH)], ot[:])

    nc.compile()
    return nc


_CACHE = {}


def _get_nc(start: int):
    if start not in _CACHE:
        _CACHE[start] = build_nc(start)
    return _CACHE[start]


def _prep_inputs(X, base_freq, Wqd, bqd, gq, Wqu, bqu, Wkv, bkv, gkv,
                 Wkvu, bkvu, Wo, bo, start):
    f = np.float32
    X = np.asarray(X, f)
    base_freq = np.asarray(base_freq, f)
    Wqd = np.asarray(Wqd, f); bqd = np.asarray(bqd, f)
    gq = np.asarray(gq, f); Wqu = np.asarray(Wqu, f); bqu = np.asarray(bqu, f)
    Wkv = np.asarray(Wkv, f); bkv = np.asarray(bkv, f)
    gkv = np.asarray(gkv, f); Wkvu = np.asarray(Wkvu, f)
    bkvu = np.asarray(bkvu, f)
    Wo = np.asarray(Wo, f); bo = np.asarray(bo, f)
    start = int(np.asarray(start).item())
    assert start >= 0

    scale = QKH ** (-0.5)
    bf = ml_dtypes.bfloat16

    # v-bias exact fold: probs sum to 1, so the v bias contributes
    # Wo @ bv to every token's output.
    bv = bkvu.reshape(H, NOPE + VH)[:, NOPE:].reshape(H * VH)
    bo_eff = bo + Wo @ bv

    def _sw(wt, nt, c, m):
        # [c*P, nt*m] -> partition-major tiles [P, nt*c*m]
        a = np.asarray(wt, f).reshape(c, P, nt, m)
        a = np.ascontiguousarray(a.transpose(1, 2, 0, 3)).astype(bf)
        return a.reshape(P, nt * c * m)

    # full down-projection weights (token-split: same on both cores)
    wqd_t = _sw(Wqd.T, QC, DC, P)
    bqd_t = np.ascontiguousarray(bqd.reshape(QC, P).T)        # (P, 12)
    wkv_t = Wkv.T.astype(f)                                   # (D, NKV)
    wkvd = _sw(wkv_t[:, :512], 4, DC, P)
    wkv5 = _sw(wkv_t[:, 512:576], 1, DC, ROPE)
    bkvd_p = np.zeros((5 * P,), f); bkvd_p[:NKV] = bkv
    bkvd = np.ascontiguousarray(bkvd_p.reshape(5, P).T)

    ang = base_freq[:S]                                       # (S, ROPE)
    cos = np.ascontiguousarray(np.cos(ang).T.astype(f))       # (ROPE, S)
    sin = np.ascontiguousarray(np.sin(ang).T.astype(f))
    cos2 = np.ascontiguousarray(
        np.concatenate([cos, cos], 0)).astype(bf)             # (128, S)
    sgn = np.ones((ROPE, 1), f); sgn[:ROPE // 2] = -1.0
    sins = sin * sgn                                          # sign-folded
    sina = np.ascontiguousarray(np.concatenate([sins, sins], 0)).astype(bf)

    # universal diagonal-band mask: for the block at k = kc*P + p,
    # q = (kc*P - start) + j, visibility is p <= j.
    pp = np.arange(P)
    tri = np.where(pp[:, None] <= pp[None, :], 0.0, NEG).astype(bf)
    tri = np.ascontiguousarray(tri)

    # per head-group tensors
    perm_q = np.concatenate(
        [np.arange(h * QKH, h * QKH + NOPE) for h in range(HH)]
        + [np.arange(h * QKH + NOPE, (h + 1) * QKH) for h in range(HH)]
    )
    perm_kv = np.concatenate(
        [np.arange(h * (NOPE + VH), h * (NOPE + VH) + NOPE) for h in range(HH)]
        + [np.arange(h * (NOPE + VH) + NOPE, (h + 1) * (NOPE + VH))
           for h in range(HH)]
    )
    gmaps = []
    for g in range(2):
        rq_ = slice(g * HH * QKH, (g + 1) * HH * QKH)
        rkv_ = slice(g * HH * (NOPE + VH), (g + 1) * HH * (NOPE + VH))
        wqu_g = (Wqu[rq_, :] * gq[None, :] * scale)[perm_q]   # (1536, QL)
        bqu_g = (bqu[rq_] * scale)[perm_q]
        wkvu_g = (Wkvu[rkv_, :] * gkv[None, :])[perm_kv]      # (2048, KVL)
        bkvu_g = bkvu[rkv_][perm_kv]
        wo_g = Wo[:, g * HH * VH:(g + 1) * HH * VH]           # (D, 1024)
        tg = slice(g * TH, (g + 1) * TH)
        wkvu_t = wkvu_g.T                                     # (KVL, 2048)
        gmaps.append({
            "wqu": _sw(wqu_g.T, QC, QC, P),
            "bqu": np.ascontiguousarray(bqu_g.reshape(QC, P).T),
            "wkn": _sw(wkvu_t[:, :HH * P], HH, KC, P),
            "wv": _sw(wkvu_t[:, HH * P:], 4, KC, 256),
            "bkvuk": np.ascontiguousarray(
                bkvu_g[:HH * NOPE].reshape(HH, P).T),
            "wo": _sw(wo_g.T, DC, HH, P),
            "cosk": np.ascontiguousarray(cos[:, tg]).astype(bf),
            "sink": np.ascontiguousarray(sins[:, tg]).astype(bf),
        })

    xts = [[_sw(X[b].T[:, g * TH:(g + 1) * TH], 1, DC, TH)
            for g in range(2)] for b in range(B)]

    in_maps = []
    for c in range(8):
        b, g = c // 2, c % 2
        m = {
            "xt": xts[b][g], "wqd": wqd_t, "bqd": bqd_t,
            "wkvd": wkvd, "wkv5": wkv5, "bkvd": bkvd,
            "cos2": cos2, "sina": sina, "tri": tri,
        }
        m.update(gmaps[g])
        in_maps.append(m)
    return in_maps, bo_eff, start


def kernel(**inputs) -> np.ndarray:
    in_maps, bo_eff, start = _prep_inputs(**inputs)
    nc = _get_nc(start)
    try:
        res = run_bass_kernel_spmd(nc, in_maps, core_ids=list(range(8)))
    except Exception:
        res = run_bass_kernel_spmd(nc, in_maps, core_ids=list(range(8)))
    out = np.empty((B, S, D), np.float32)
    for b in range(B):
        acc = (res.results[2 * b]["outt"].astype(np.float32)
               + res.results[2 * b + 1]["outt"].astype(np.float32))
        out[b] = acc.T + bo_eff[None, :]
    return out
